# revision 1
# baseline (speedup 1.0000x reference)
"""Trainium2 Bass kernel for ConvMessageAggregator.

Computes, for each node n (messages: [N, 16, 688] fp32):
  f1[i] = relu(w10*x[i] + w11*x[i+2] + b1)      i in 0..13   (dilated 2-tap conv)
  f2[i] = relu(w20*f1[i] + w21*f1[i+2] + b2)    i in 0..11
  out   = relu(sum_k mlp_w[k] * f2[6+k] + mlp_b)             -> [N, 688]

Only f2 rows 6..11 are consumed, which depend on f1 rows 6..13, which depend
on x rows 6..15 -- so the kernel only reads the last 10 (contiguous) rows of
each node's 16-row block (10/16 of the input bytes).

Sharding: pure data parallel; node axis split across 8 NeuronCores, all
conv/MLP scalars baked into the instruction stream as immediates at trace
time (the program is rebuilt per call, so arbitrary weights are handled).

Per-core pipeline (2048 nodes = 16 tiles of 128 nodes on partitions):
  DMA  x[128, 10, 688]                                (HWDGE, one 3.5MB DMA)
  DVE  u1 = (x_other * r1) + x_pivot    [128, 8, 688] (scalar_tensor_tensor)
  ACT  f1 = Relu(p1*u1 + b1)            [128, 8, 688]
  DVE  u2 = (f1_other * r2) + f1_pivot  [128, 6, 688]
  ACT  f2 = Relu(p2*u2 + b2)            [128, 6, 688]
  DVE  5x binary-combine tree over the 6 rows (weight ratios all <= 1)
  ACT  out = Relu(w_anchor*t + mlp_b)   [128, 688]
  DMA  out tile -> DRAM
"""

import sys

for _p in ("/opt/trn_rl_repo",):
    if _p not in sys.path:
        sys.path.insert(0, _p)

import numpy as np

import concourse.bass as bass
import concourse.tile as tile
from concourse import mybir
from concourse.bass_utils import run_bass_kernel_spmd

N_FULL, L, MSG = 16384, 16, 688
N_CORES = 8
N_LOCAL = N_FULL // N_CORES  # 2048
P = 128                      # nodes per tile (partition dim)
NTILES = N_LOCAL // P        # 16
R0, NROWS = 6, 10            # input rows actually used: 6..15 (contiguous)

F32 = mybir.dt.float32
AF = mybir.ActivationFunctionType
OP = mybir.AluOpType


def _split_multi_waits(nc):
    """TPB instructions encode at most ONE semaphore wait; this walrus build's
    codegen rejects instructions with more. Hoist extra waits into standalone
    EventSemaphore ops on the same (in-order) sequencer -- semantically
    identical to the attached wait."""
    for func in nc.m.functions:
        for bb in func.blocks:
            insts = list(bb.instructions)
            if not any(
                i.sync_info is not None and len(i.sync_info.on_wait) > 1
                for i in insts
            ):
                continue
            new = []
            for inst in insts:
                si = inst.sync_info
                if si is not None and len(si.on_wait) > 1:
                    waits = list(si.on_wait)
                    for j, w in enumerate(waits[:-1]):
                        new.append(
                            mybir.InstEventSemaphore(
                                name=f"{inst.name}-hoistw{j}",
                                engine=inst.engine,
                                sync_info=mybir.SyncInfo(on_wait=[w], on_update=[]),
                            )
                        )
                    inst.sync_info = mybir.SyncInfo(
                        on_wait=[waits[-1]], on_update=list(si.on_update)
                    )
                new.append(inst)
            bb.instructions = new


def _conv_split(wa, wb):
    """Factor pre[i] = wa*in[i] + wb*in[i+2] as pivot*(in[pv] + r*in[ot]).

    Returns (pivot_weight, ratio, pivot_row_off, other_row_off) with |ratio|<=1.
    """
    if abs(wa) >= abs(wb):
        return wa, (wb / wa if wa != 0.0 else 0.0), 0, 2
    return wb, wa / wb, 2, 0


def build_program(w10, w11, b1, w20, w21, b2, mlp_w, mlp_b):
    nc = bass.Bass(trn_type="TRN2", name="conv_msg_agg")
    x = nc.dram_tensor("x", [N_LOCAL, L, MSG], F32, kind="ExternalInput")
    out = nc.dram_tensor("out", [N_LOCAL, MSG], F32, kind="ExternalOutput")

    p1, r1, pv1, ot1 = _conv_split(w10, w11)
    p2, r2, pv2, ot2 = _conv_split(w20, w21)

    # mlp weighted-sum plan: anchor a = argmax |mlp_w|.  For each nonzero k,
    # G[k] = s_k * relu(conv2[k]) with s_k = |mlp_w[k]/mlp_w[a]| <= 1, computed
    # in ONE ACT op from u2 (relu(s_k*p2*u2[k] + s_k*b2)).  Then
    # sum = mlp_w[a] * sum_k tau_k G[k] (tau_k = sign) via plain add/sub
    # tensor_tensor ops on the otherwise-idle GPSIMD engine (walrus rejects
    # TensorScalarPtr on Pool, so the tree must be scalar-free).
    nzk = [k for k in range(6) if mlp_w[k] != 0.0]
    anchor = max(nzk, key=lambda k: abs(mlp_w[k])) if nzk else -1
    wa = mlp_w[anchor] if nzk else 0.0

    with tile.TileContext(nc) as tc:
        with (
            tc.tile_pool(name="bias", bufs=1) as pool_b,
            tc.tile_pool(name="xin", bufs=2) as pool_x,
            tc.tile_pool(name="work", bufs=2) as pool_w,
            tc.tile_pool(name="gbuf", bufs=2) as pool_g,
            tc.tile_pool(name="outp", bufs=2) as pool_o,
        ):
            # activation() needs SBUF [P,1] bias vectors for non-Copy funcs
            b1c = pool_b.tile([P, 1], F32, tag="b1")
            nc.vector.memset(b1c[:], b1)
            gbias = {}
            for k in nzk:
                s_k = abs(mlp_w[k] / wa)
                gbias[k] = pool_b.tile([P, 1], F32, tag=f"gb{k}", name=f"gb{k}")
                nc.vector.memset(gbias[k][:], s_k * b2)
            mbc = pool_b.tile([P, 1], F32, tag="mb")
            nc.vector.memset(mbc[:], mlp_b)

            for it in range(NTILES):
                n0 = it * P
                xt = pool_x.tile([P, NROWS, MSG], F32, tag="x")
                nc.gpsimd.dma_start(out=xt[:], in_=x[n0 : n0 + P, R0 : R0 + NROWS, :])

                # conv1: u1 = x_pv + r1*x_ot (DVE), then relu-affine IN PLACE
                # (ACT) -- saves 22KB/partition so the chain double-buffers
                u1 = pool_w.tile([P, 8, MSG], F32, tag="u1")
                if p1 == 0.0:
                    nc.vector.memset(u1[:], max(b1, 0.0))
                else:
                    nc.vector.scalar_tensor_tensor(
                        out=u1[:],
                        in0=xt[:, ot1 : ot1 + 8, :],
                        scalar=r1,
                        in1=xt[:, pv1 : pv1 + 8, :],
                        op0=OP.mult,
                        op1=OP.add,
                    )
                    nc.scalar.activation(
                        out=u1[:], in_=u1[:], func=AF.Relu, bias=b1c[:], scale=p1
                    )

                # conv2 pre-activation (DVE)
                u2 = pool_w.tile([P, 6, MSG], F32, tag="u2")
                if p2 == 0.0:
                    nc.vector.memset(u2[:], 0.0)
                    u2_scale, u2_bias_val = 0.0, b2
                else:
                    nc.vector.scalar_tensor_tensor(
                        out=u2[:],
                        in0=u1[:, ot2 : ot2 + 6, :],
                        scalar=r2,
                        in1=u1[:, pv2 : pv2 + 6, :],
                        op0=OP.mult,
                        op1=OP.add,
                    )
                    u2_scale, u2_bias_val = p2, b2

                # G[k] = s_k*relu(conv2[k]) fused into one ACT op per row
                terms = []  # (tau, tile_ap)
                for k in nzk:
                    s_k = abs(mlp_w[k] / wa)
                    gk = pool_g.tile([P, MSG], F32, tag=f"g{k}", name=f"g{k}")
                    nc.scalar.activation(
                        out=gk[:],
                        in_=u2[:, k, :],
                        func=AF.Relu,
                        bias=gbias[k][:],
                        scale=s_k * u2_scale,
                    )
                    terms.append((1 if mlp_w[k] / wa > 0 else -1, gk[:]))

                # scalar-free signed combine tree on GPSIMD, in-place into
                # the left operand's tile
                while len(terms) > 1:
                    nxt = []
                    for i in range(0, len(terms) - 1, 2):
                        ta, aa = terms[i]
                        tb, ab = terms[i + 1]
                        op = OP.add if ta == tb else OP.subtract
                        nc.gpsimd.tensor_tensor(out=aa, in0=aa, in1=ab, op=op)
                        nxt.append((ta, aa))
                    if len(terms) % 2:
                        nxt.append(terms[-1])
                    terms = nxt

                ot = pool_o.tile([P, MSG], F32, tag="o")
                if terms:
                    tau, aa = terms[0]
                    nc.scalar.activation(
                        out=ot[:], in_=aa, func=AF.Relu, bias=mbc[:], scale=wa * tau
                    )
                else:
                    nc.vector.memset(ot[:], max(mlp_b, 0.0))
                nc.gpsimd.dma_start(out=out[n0 : n0 + P, :], in_=ot[:])
    _split_multi_waits(nc)
    return nc


def run(inputs, trace=False, **spmd_kwargs):
    """Build + run on 8 cores. Returns (full_output, BassKernelResults)."""
    msgs = np.asarray(inputs["messages"], dtype=np.float32)
    assert msgs.shape == (N_FULL, L, MSG), msgs.shape
    if not msgs.flags["C_CONTIGUOUS"]:
        msgs = np.ascontiguousarray(msgs)

    c1w = np.asarray(inputs["conv1_w"], dtype=np.float64)
    c2w = np.asarray(inputs["conv2_w"], dtype=np.float64)
    mlw = np.asarray(inputs["mlp_w"], dtype=np.float64)
    nc = build_program(
        float(c1w[0]),
        float(c1w[1]),
        float(np.asarray(inputs["conv1_b"], dtype=np.float64)),
        float(c2w[0]),
        float(c2w[1]),
        float(np.asarray(inputs["conv2_b"], dtype=np.float64)),
        [float(v) for v in mlw],
        float(np.asarray(inputs["mlp_b"], dtype=np.float64)),
    )

    in_maps = [
        {"x": msgs[i * N_LOCAL : (i + 1) * N_LOCAL]} for i in range(N_CORES)
    ]
    res = run_bass_kernel_spmd(
        nc, in_maps, core_ids=list(range(N_CORES)), trace=trace, **spmd_kwargs
    )
    full = np.concatenate([r["out"] for r in res.results], axis=0)
    return full, res


def kernel(**inputs) -> np.ndarray:
    return run(inputs, trace=False)[0]



# revision 4
# speedup vs baseline: 1.2456x; 1.2456x over previous
"""Trainium2 Bass kernel for ConvMessageAggregator (v2).

Computes, for each node n (messages: [N, 16, 688] fp32):
  f1[i] = relu(w10*x[i] + w11*x[i+2] + b1)      i in 0..13   (dilated 2-tap conv)
  f2[i] = relu(w20*f1[i] + w21*f1[i+2] + b2)    i in 0..11
  out   = relu(sum_k mlp_w[k] * f2[6+k] + mlp_b)             -> [N, 688]

Only f2 rows 6..11 are consumed -> only x rows 6..15 (10 contiguous rows) are
read per node.

Sharding: pure data parallel across 8 NeuronCores; all scalars baked as
immediates at trace time (program rebuilt per call).

v2 redesign (from the v1 trace: Pool TENSOR_TENSOR tree 246us, DVE STT 214us,
ACT 174us vs a ~170us DMA roofline):
  * The only fp32 STT (no DVE fast modes exist for STT) is conv1's pre-act;
    everything downstream is bf16.
  * relu1 runs as a DVE tensor_scalar in max-form -- m=(u1+b1/p1) max 0, the
    pivot scale p1 folded into later stages -- hitting the 4x 2-byte DVE mode.
  * conv2 splits into an ACT copy-scale (y=r2*m) + a bf16 DVE tensor_tensor
    add (2x mode), balancing DVE vs ACT.
  * relu2 + the mlp row weights fuse into 6 ACT relus g_k=relu(|w_k|P2*u2k
    + |w_k|b2) written into sign-sorted row slots; the weighted sum becomes
    an unweighted halving add-tree (+ one subtract), in cheap bf16 ops split
    between DVE's leftover budget and the otherwise-idle Pool engine.
  * output is written bf16 (halves write traffic); host upcasts.

Per-core model (16 tiles of 128 nodes): DVE ~168us, ACT ~151us, Pool ~125us,
DMA ~160us -> ~1.8x over v1's 334us.
"""

import sys

for _p in ("/opt/trn_rl_repo",):
    if _p not in sys.path:
        sys.path.insert(0, _p)

import numpy as np

import concourse.bass as bass
import concourse.tile as tile
from concourse import mybir
from concourse.bass_utils import run_bass_kernel_spmd

N_FULL, L, MSG = 16384, 16, 688
N_CORES = 8
N_LOCAL = N_FULL // N_CORES  # 2048
P = 128                      # nodes per tile (partition dim)
NTILES = N_LOCAL // P        # 16
R0, NROWS = 6, 10            # input rows actually used: 6..15 (contiguous)

F32 = mybir.dt.float32
BF16 = mybir.dt.bfloat16
AF = mybir.ActivationFunctionType
OP = mybir.AluOpType

# rough per-op cost estimates (ns) used to split the add-tree between DVE's
# leftover budget and Pool
DVE_TREE_BUDGET_NS = 950.0


def _split_multi_waits(nc):
    """TPB instructions encode at most ONE semaphore wait; this walrus build's
    codegen rejects instructions with more. Hoist extra waits into standalone
    EventSemaphore ops on the same (in-order) sequencer -- semantically
    identical to the attached wait."""
    for func in nc.m.functions:
        for bb in func.blocks:
            insts = list(bb.instructions)
            if not any(
                i.sync_info is not None and len(i.sync_info.on_wait) > 1
                for i in insts
            ):
                continue
            new = []
            for inst in insts:
                si = inst.sync_info
                if si is not None and len(si.on_wait) > 1:
                    waits = list(si.on_wait)
                    for j, w in enumerate(waits[:-1]):
                        new.append(
                            mybir.InstEventSemaphore(
                                name=f"{inst.name}-hoistw{j}",
                                engine=inst.engine,
                                sync_info=mybir.SyncInfo(on_wait=[w], on_update=[]),
                            )
                        )
                    inst.sync_info = mybir.SyncInfo(
                        on_wait=[waits[-1]], on_update=list(si.on_update)
                    )
                new.append(inst)
            bb.instructions = new


def _conv_split(wa, wb):
    """Factor pre[i] = wa*in[i] + wb*in[i+2] as pivot*(in[pv] + r*in[ot]).

    Returns (pivot_weight, ratio, pivot_row_off, other_row_off) with |ratio|<=1.
    """
    if abs(wa) >= abs(wb):
        return wa, (wb / wa if wa != 0.0 else 0.0), 0, 2
    return wb, wa / wb, 2, 0


def _tree_ops(p, q):
    """Plan the in-place row-sum tree over gbuf rows [0:p] (pos) and
    [p:p+q] (neg), plus the final subtract. Returns a list of
    (lo, rows, rhs_lo, alu_op) acting on gbuf row slices:
        gbuf[:, lo:lo+rows, :] op= gbuf[:, rhs_lo:rhs_lo+rows, :]
    After all ops, the result sum(pos) - sum(neg) lives in row 0 (if p>0)
    else -result in row 0."""
    ops = []

    def halving(lo, n):
        while n > 1:
            h = n // 2
            ops.append((lo, h, lo + h, OP.add))
            if n % 2:
                ops.append((lo, 1, lo + 2 * h, OP.add))
            n = h

    halving(0, p)
    halving(p, q)
    if p > 0 and q > 0:
        ops.append((0, 1, p, OP.subtract))
    return ops


def build_program(w10, w11, b1, w20, w21, b2, mlp_w, mlp_b):
    nc = bass.Bass(trn_type="TRN2", name="conv_msg_agg")
    x = nc.dram_tensor("x", [N_LOCAL, L, MSG], F32, kind="ExternalInput")
    out = nc.dram_tensor("out", [N_LOCAL, MSG], BF16, kind="ExternalOutput")

    p1, r1, pv1, ot1 = _conv_split(w10, w11)
    p2, r2, pv2, ot2 = _conv_split(w20, w21)
    nzk = [k for k in range(6) if mlp_w[k] != 0.0]

    # constant-output degenerate cases (any conv layer all-zero, or mlp zero)
    if p1 == 0.0 or p2 == 0.0 or not nzk:
        if p1 == 0.0:
            f1c = max(b1, 0.0)
            f2c = max((w20 + w21) * f1c + b2, 0.0)
        elif p2 == 0.0:
            f2c = max(b2, 0.0)
        else:
            f2c = 0.0  # unused (nzk empty)
        cval = max(sum(mlp_w[k] for k in range(6)) * f2c + mlp_b, 0.0)
        with tile.TileContext(nc) as tc:
            with tc.tile_pool(name="outp", bufs=1) as pool_o:
                ot = pool_o.tile([P, MSG], BF16, tag="o")
                nc.vector.memset(ot[:], cval)
                for it in range(NTILES):
                    n0 = it * P
                    nc.gpsimd.dma_start(out=out[n0 : n0 + P, :], in_=ot[:])
        _split_multi_waits(nc)
        return nc

    # scalar folding:
    #   f1 = relu(p1*u1 + b1) = p1 * m,  m = (u1 + b1/p1) extremum-with-0
    c1 = b1 / p1
    m_op = OP.max if p1 > 0 else OP.min
    P2 = p1 * p2  # z2 = P2*u2 + b2

    # g_k = |w_k| * relu(z2_k) = relu(|w_k|*P2*u2k + |w_k|*b2), sign-sorted
    kpos = [k for k in nzk if mlp_w[k] > 0]
    kneg = [k for k in nzk if mlp_w[k] < 0]
    slot_of = {}
    for j, k in enumerate(kpos + kneg):
        slot_of[k] = j
    np_, nq = len(kpos), len(kneg)

    tree = _tree_ops(np_, nq)
    # assign tree ops: biggest to DVE while it has budget, rest to Pool
    order = sorted(range(len(tree)), key=lambda i: -tree[i][1])
    dve_left = DVE_TREE_BUDGET_NS
    on_dve = set()
    for i in order:
        rows = tree[i][1]
        c = rows * MSG * 0.52 + 90.0
        if c <= dve_left:
            on_dve.add(i)
            dve_left -= c

    final_scale = 1.0 if np_ > 0 else -1.0

    with tile.TileContext(nc) as tc:
        with (
            tc.tile_pool(name="bias", bufs=1) as pool_b,
            tc.tile_pool(name="xin", bufs=2) as pool_x,
            tc.tile_pool(name="u1p", bufs=2) as pool_u1,
            tc.tile_pool(name="mp", bufs=2) as pool_m,
            tc.tile_pool(name="yp", bufs=2) as pool_y,
            tc.tile_pool(name="u2p", bufs=2) as pool_u2,
            tc.tile_pool(name="gp", bufs=2) as pool_g,
            tc.tile_pool(name="outp", bufs=2) as pool_o,
        ):
            # activation() needs SBUF [P,1] bias vectors for non-Copy funcs
            gbias = {}
            for k in nzk:
                gbias[k] = pool_b.tile([P, 1], F32, tag=f"gb{k}", name=f"gb{k}")
                nc.vector.memset(gbias[k][:], abs(mlp_w[k]) * b2)
            mbc = pool_b.tile([P, 1], F32, tag="mb")
            nc.vector.memset(mbc[:], mlp_b)

            for it in range(NTILES):
                n0 = it * P
                xt = pool_x.tile([P, NROWS, MSG], F32, tag="x")
                nc.gpsimd.dma_start(out=xt[:], in_=x[n0 : n0 + P, R0 : R0 + NROWS, :])

                # conv1 pre-act (the one unavoidable fp32 STT), bf16 out
                u1 = pool_u1.tile([P, 8, MSG], BF16, tag="u1")
                nc.vector.scalar_tensor_tensor(
                    out=u1[:],
                    in0=xt[:, ot1 : ot1 + 8, :],
                    scalar=r1,
                    in1=xt[:, pv1 : pv1 + 8, :],
                    op0=OP.mult,
                    op1=OP.add,
                )

                # relu1 in max-form: m = (u1 + c1) max/min 0   (DVE 4x bf16)
                m = pool_m.tile([P, 8, MSG], BF16, tag="m")
                nc.vector.tensor_scalar(m[:], u1[:], c1, 0.0, OP.add, m_op)

                # conv2 pre-act: y = r2*m[ot]  (ACT), u2 = m[pv] + y (DVE 2x)
                y = pool_y.tile([P, 6, MSG], BF16, tag="y")
                nc.scalar.activation(
                    out=y[:], in_=m[:, ot2 : ot2 + 6, :], func=AF.Copy, scale=r2
                )
                u2 = pool_u2.tile([P, 6, MSG], BF16, tag="u2")
                nc.vector.tensor_tensor(
                    out=u2[:], in0=m[:, pv2 : pv2 + 6, :], in1=y[:], op=OP.add
                )

                # g_k = relu(|w_k|*P2*u2k + |w_k|*b2) into sign-sorted slots
                g = pool_g.tile([P, 6, MSG], BF16, tag="g")
                for k in nzk:
                    aw = abs(mlp_w[k])
                    nc.scalar.activation(
                        out=g[:, slot_of[k], :],
                        in_=u2[:, k, :],
                        func=AF.Relu,
                        scale=aw * P2,
                        bias=gbias[k][:],
                    )

                # halving add-tree (+ subtract), split DVE / Pool
                for i, (lo, rows, rhs, op) in enumerate(tree):
                    eng = nc.vector if i in on_dve else nc.gpsimd
                    eng.tensor_tensor(
                        out=g[:, lo : lo + rows, :],
                        in0=g[:, lo : lo + rows, :],
                        in1=g[:, rhs : rhs + rows, :],
                        op=op,
                    )

                # out = relu(sign * S + mlp_b), bf16
                ot = pool_o.tile([P, MSG], BF16, tag="o")
                nc.scalar.activation(
                    out=ot[:],
                    in_=g[:, 0, :],
                    func=AF.Relu,
                    scale=final_scale,
                    bias=mbc[:],
                )
                nc.gpsimd.dma_start(out=out[n0 : n0 + P, :], in_=ot[:])
    _split_multi_waits(nc)
    return nc


def run(inputs, trace=False, **spmd_kwargs):
    """Build + run on 8 cores. Returns (full_output, BassKernelResults)."""
    msgs = np.asarray(inputs["messages"], dtype=np.float32)
    assert msgs.shape == (N_FULL, L, MSG), msgs.shape
    if not msgs.flags["C_CONTIGUOUS"]:
        msgs = np.ascontiguousarray(msgs)

    c1w = np.asarray(inputs["conv1_w"], dtype=np.float64)
    c2w = np.asarray(inputs["conv2_w"], dtype=np.float64)
    mlw = np.asarray(inputs["mlp_w"], dtype=np.float64)
    nc = build_program(
        float(c1w[0]),
        float(c1w[1]),
        float(np.asarray(inputs["conv1_b"], dtype=np.float64)),
        float(c2w[0]),
        float(c2w[1]),
        float(np.asarray(inputs["conv2_b"], dtype=np.float64)),
        [float(v) for v in mlw],
        float(np.asarray(inputs["mlp_b"], dtype=np.float64)),
    )

    in_maps = [
        {"x": msgs[i * N_LOCAL : (i + 1) * N_LOCAL]} for i in range(N_CORES)
    ]
    res = run_bass_kernel_spmd(
        nc, in_maps, core_ids=list(range(N_CORES)), trace=trace, **spmd_kwargs
    )
    full = np.concatenate(
        [np.asarray(r["out"], dtype=np.float32) for r in res.results], axis=0
    )
    return full, res


def kernel(**inputs) -> np.ndarray:
    return run(inputs, trace=False)[0]
